# revision 1
# baseline (speedup 1.0000x reference)
"""Trainium2 Bass kernel for nn_EnhancedHybridModel.

Pipeline per core (pure data parallel over batch, 128 images/core):
  conv1(3->32,3x3,p1)+BN+ReLU -> maxpool2 -> conv2(32->64)+BN+ReLU -> maxpool2
  -> conv3(64->128)+BN+ReLU -> avgpool2 -> fc 2048->512 -> fc 512->16
  -> softmax -> 4-qubit statevector sim (collapses to two fixed real 16x16
  matmuls built on host from q_weights) -> head 4->128->100.

Conv strategy: channels on partitions, im2col K-packing, dy via free-dim
shifted accumulating matmuls, images column-packed into the PE array via
tile_position so pooling runs on up to 128 DVE lanes.  Conv/fc1 matmul
operands are fp16 (1 cycle/row on the PE, fp32 PSUM accumulate); everything
from fc2 on is exact fp32.  The quantum layer is data-independent given
q_weights, so it folds into two real 16x16 matmuls + one reciprocal (U is
unitary, so the L2 normalization needs no sqrt).
"""

import numpy as np

NB = 128          # images per core
NCORES = 8
HP1 = 34          # padded conv1 input grid
ROWL = 1160       # padded im2col row length per image (1156 + slack)
EPS = 1e-5

_cache = {}


# ---------------------------------------------------------------------------
# host-side math (quantum layer constants, weight folding, im2col rows)
# ---------------------------------------------------------------------------

def _cnot_ring_matrix():
    M = np.zeros((16, 16), dtype=np.complex64)
    for b in range(16):
        bb = b
        for cw, tw in [(0, 1), (1, 2), (2, 3), (3, 0)]:
            if (bb >> (3 - cw)) & 1:
                bb ^= 1 << (3 - tw)
        M[bb, b] = 1.0
    return M


def _zsigns():
    return np.array([[1.0 - 2.0 * ((b >> (3 - w)) & 1) for b in range(16)]
                     for w in range(4)], dtype=np.float32)


def _quantum_unitary(q_weights):
    CN = _cnot_ring_matrix()
    U_tot = np.eye(16, dtype=np.complex64)
    for l in range(2):
        c = np.cos(q_weights[l] * 0.5).astype(np.complex64)
        s = np.sin(q_weights[l] * 0.5).astype(np.complex64)
        U = np.ones((1, 1), dtype=np.complex64)
        for q in range(4):
            g = np.array([[c[q], -1j * s[q]], [-1j * s[q], c[q]]], dtype=np.complex64)
            U = np.kron(U, g)
        U_tot = (CN @ U) @ U_tot
    return U_tot  # psi_out = psi_in @ U_tot.T


def _host_weights(inp):
    f32, f16 = np.float32, np.float16
    sc = f32(1.0 / np.sqrt(1.0 + EPS))
    out = {}

    g1 = inp['bn1_g'] * sc
    w1 = np.zeros((27, 32), f32)
    for dy in range(3):
        for dx in range(3):
            for ci in range(3):
                w1[(dy * 3 + dx) * 3 + ci, :] = inp['conv1_w'][:, ci, dy, dx] * g1
    w1r = np.zeros((128, 32), f16)
    for blk in range(4):
        w1r[32 * blk:32 * blk + 27] = w1
    out['W1R'] = w1r
    b1 = inp['conv1_b'] * g1 + inp['bn1_b']
    out['B1R'] = np.tile(b1, 4)[:, None].astype(f32)

    g2 = inp['bn2_g'] * sc
    w2 = np.zeros((96, 192), f32)
    for dy in range(3):
        for dx in range(3):
            for ci in range(32):
                w2[dx * 32 + ci, dy * 64:(dy + 1) * 64] = inp['conv2_w'][:, ci, dy, dx] * g2
    out['W2'] = w2.astype(f16)
    out['B2'] = (inp['conv2_b'] * g2 + inp['bn2_b'])[:, None].astype(f32)

    g3 = inp['bn3_g'] * sc
    w3a = np.zeros((128, 384), f32)
    w3b = np.zeros((64, 384), f32)
    for dy in range(3):
        for ci in range(64):
            for dx in range(2):
                w3a[dx * 64 + ci, dy * 128:(dy + 1) * 128] = inp['conv3_w'][:, ci, dy, dx] * g3
            w3b[ci, dy * 128:(dy + 1) * 128] = inp['conv3_w'][:, ci, dy, 2] * g3
    out['W3A'] = w3a.astype(f16)
    out['W3B'] = w3b.astype(f16)
    out['B3'] = (inp['conv3_b'] * g3 + inp['bn3_b']).astype(f32)[:, None]

    # fc1 with avgpool folded in: input index = c*16 + (y2*4+x2), pool = 0.25*sum
    fr1 = inp['fr1_w'].reshape(512, 128, 16)  # [m, c, s]
    w1fc = np.zeros((128, 16 * 512), f32)
    for s in range(16):
        w1fc[:, s * 512:(s + 1) * 512] = (fr1[:, :, s].T * 0.25)
    out['W1FC'] = w1fc.astype(f16)
    out['B1FC'] = inp['fr1_b'].astype(f32)[None, :]

    fr2 = inp['fr2_w']  # [16, 512]
    w2fc = np.zeros((128, 64), f32)
    for t in range(4):
        w2fc[:, t * 16:(t + 1) * 16] = fr2[:, t * 128:(t + 1) * 128].T
    out['W2FC'] = w2fc
    out['B2FC'] = inp['fr2_b'].astype(f32)[:, None]

    U = _quantum_unitary(np.asarray(inp['q_weights'], np.float64))
    out['URT'] = np.ascontiguousarray(np.real(U).T.astype(f32))   # [i, j] = Re(U)[j, i]
    out['UIT'] = np.ascontiguousarray(np.imag(U).T.astype(f32))

    ZS = _zsigns()
    out['WH'] = np.ascontiguousarray((inp['h1_w'] @ ZS).T.astype(f32))  # [16j, 128m]
    ah = inp['bnh_g'] * sc
    out['AH'] = ah.astype(f32)[:, None]
    out['CH'] = (ah * inp['h1_b'] + inp['bnh_b']).astype(f32)[:, None]

    out['H2WT'] = np.ascontiguousarray(inp['h2_w'].T.astype(f32))  # [128, 100]
    out['H2B'] = inp['h2_b'].astype(f32)[None, :]
    return out


def _build_xr(x):
    """Host im2col rows for conv1, packed as [128, G8*2*ROWL]: partition
    32*blk + r holds row r=(dy*3+dx)*3+ci of images (8g + 2blk + {0,1}) at
    columns (g, w).  One clean 2D DMA loads a whole 8-image chunk."""
    B = x.shape[0]
    G8 = B // 8
    xp = np.zeros((B, 3, HP1 * HP1 + 3 * HP1), np.float16)  # flat plane + slack
    xpv = xp[:, :, :HP1 * HP1].reshape(B, 3, HP1, HP1)
    xpv[:, :, 1:33, 1:33] = x
    xr = np.zeros((27, B, ROWL), np.float16)
    for dy in range(3):
        for dx in range(3):
            sh = dy * HP1 + dx
            for ci in range(3):
                xr[(dy * 3 + dx) * 3 + ci, :, :] = xp[:, ci, sh:sh + ROWL]
    # [27, B, ROWL] -> [27, G8, blk4, w2, ROWL] -> [blk4, 27, G8, w2, ROWL]
    xrb = xr.reshape(27, G8, 4, 2, ROWL).transpose(2, 0, 1, 3, 4)
    xr2 = np.zeros((4, 32, G8, 2, ROWL), np.float16)
    xr2[:, :27] = xrb
    return np.ascontiguousarray(xr2.reshape(128, G8 * 2 * ROWL))


# ---------------------------------------------------------------------------
# device program
# ---------------------------------------------------------------------------

def _build_program(nb):
    import concourse.bass as bass
    import concourse.tile as tile
    from concourse import bacc, mybir
    from concourse.masks import make_identity
    from contextlib import ExitStack

    f32 = mybir.dt.float32
    f16 = mybir.dt.float16
    AF = mybir.ActivationFunctionType
    ALU = mybir.AluOpType
    AX = mybir.AxisListType

    def view(base_ap, part_start, nparts, free_off, free_dims):
        pitch = base_ap.ap[0][0]
        return bass.AP(tensor=base_ap.tensor,
                       offset=base_ap.offset + part_start * pitch + free_off,
                       ap=[[pitch, nparts]] + [list(d) for d in free_dims])

    nc = bacc.Bacc("TRN2", target_bir_lowering=False)
    G8 = nb // 8

    XR = nc.declare_dram_parameter("xr", [128, (nb // 8) * 2 * ROWL], f16, isOutput=False)
    dparams = {}
    for name, shape, dt in [("W1R", [128, 32], f16), ("B1R", [128, 1], f32),
                            ("W2", [96, 192], f16), ("B2", [64, 1], f32),
                            ("W3A", [128, 384], f16), ("W3B", [64, 384], f16),
                            ("B3", [128, 1], f32),
                            ("W1FC", [128, 16 * 512], f16), ("B1FC", [1, 512], f32),
                            ("W2FC", [128, 64], f32), ("B2FC", [16, 1], f32),
                            ("URT", [16, 16], f32), ("UIT", [16, 16], f32),
                            ("WH", [16, 128], f32), ("AH", [128, 1], f32),
                            ("CH", [128, 1], f32),
                            ("H2WT", [128, 100], f32), ("H2B", [1, 100], f32)]:
        dparams[name] = nc.declare_dram_parameter(name, shape, dt, isOutput=False)
    OUT = nc.declare_dram_parameter("out", [nb, 100], f32, isOutput=True)

    with tile.TileContext(nc) as tc, ExitStack() as ctx:
        const = ctx.enter_context(tc.tile_pool(name="const", bufs=1))
        ct = {}
        for name in dparams:
            t = const.tile(list(dparams[name].shape), dparams[name].dtype, tag="c_" + name)
            nc.sync.dma_start(t[:], dparams[name][:])
            ct[name] = t
        ident = const.tile([128, 128], f32, tag="ident")
        make_identity(nc, ident[:])
        ones_r = const.tile([1, 128], f32, tag="ones_r")
        nc.vector.memset(ones_r[:], 1.0)
        ones_c = const.tile([16, 1], f32, tag="ones_c")
        nc.vector.memset(ones_c[:], 1.0)
        ef = const.tile([128, nb * 16], f16, tag="ef")     # fc1 input accumulator

        xrap = XR[:]

        with tc.tile_pool(name="r1p", bufs=3) as r1p, \
             tc.tile_pool(name="r2p", bufs=2) as r2p, \
             tc.tile_pool(name="r3p", bufs=2) as r3p, \
             tc.tile_pool(name="t1p", bufs=3) as t1p, \
             tc.tile_pool(name="ps1p", bufs=2, space="PSUM") as ps1p, \
             tc.tile_pool(name="ps2p", bufs=2, space="PSUM") as ps2p, \
             tc.tile_pool(name="ps3p", bufs=2, space="PSUM") as ps3p:

            for g in range(G8):
                # ---- R1: host-built 27-row im2col, 4 partition blocks x 2 imgs ----
                r1 = r1p.tile([128, 2 * ROWL], f16, tag="r1")
                nc.scalar.dma_start(r1[:], xrap[:, g * 2 * ROWL:(g + 1) * 2 * ROWL])

                # ---- R2 (conv2 rhs) / R3 (conv3 rhs) with zero borders ----
                r2 = r2p.tile([96, 8 * 324 + 8], f16, tag="r2")
                nc.gpsimd.memset(view(r2[:], 0, 32, 0, [[324, 8], [306, 2], [1, 18]]), 0.0)
                nc.gpsimd.memset(view(r2[:], 0, 32, 0, [[324, 8], [18, 18], [17, 2]]), 0.0)
                nc.gpsimd.memset(r2[0:32, 8 * 324:8 * 324 + 8], 0.0)
                r3 = r3p.tile([128, 800 + 8], f16, tag="r3")
                nc.gpsimd.memset(view(r3[:], 0, 64, 0, [[100, 8], [90, 2], [1, 10]]), 0.0)
                nc.gpsimd.memset(view(r3[:], 0, 64, 0, [[100, 8], [10, 10], [9, 2]]), 0.0)
                nc.gpsimd.memset(r3[0:64, 800:808], 0.0)

                # ---- conv1 + maxpool1 (4 images column-packed) ----
                for gi in range(2):
                    ps1 = ps1p.tile([128, 1024], f32, tag="ps1")
                    for s in range(4):
                        img = 4 * gi + s
                        blk, w = img // 2, img % 2
                        for h in range(2):
                            rhs = view(r1[:], 32 * blk, 27, w * ROWL + h * 16 * HP1,
                                       [[HP1, 16], [1, 32]])
                            nc.tensor.matmul(ps1[32 * s:32 * s + 32, h * 512:(h + 1) * 512],
                                             ct['W1R'][32 * blk:32 * blk + 27, :], rhs,
                                             start=True, stop=True,
                                             tile_position=(32 * blk, 32 * s))
                    t1 = t1p.tile([128, 256], f16, tag="t1")
                    nc.vector.tensor_reduce(
                        out=t1[:],
                        in_=view(ps1[:], 0, 128, 0, [[64, 16], [2, 16], [32, 2], [1, 2]]),
                        op=ALU.max, axis=AX.XY)
                    for s in range(4):
                        img = 4 * gi + s
                        dstv = view(r2[:], 0, 32, img * 324 + 19, [[18, 16], [1, 16]])
                        srcv = view(t1[:], 32 * s, 32, 0, [[16, 16], [1, 16]])
                        eng = nc.vector if s % 2 == 0 else nc.gpsimd
                        eng.tensor_scalar(dstv, srcv, ct['B1R'][32 * s:32 * s + 32, :],
                                          0.0, op0=ALU.add, op1=ALU.max)

                # dx-shifted copies for conv2's K-packing
                nc.gpsimd.dma_start(r2[32:64, 0:8 * 324],
                                    view(r2[:], 0, 32, 1, [[1, 8 * 324]]))
                nc.sync.dma_start(r2[64:96, 0:8 * 324],
                                  view(r2[:], 0, 32, 2, [[1, 8 * 324]]))

                # ---- conv2 + maxpool2 (2 images column-packed) ----
                for p in range(4):
                    ps2 = ps2p.tile([128, 256], f32, tag="ps2")
                    for sp in range(2):
                        img = 2 * p + sp
                        for dy in range(3):
                            rhs = view(r2[:], 0, 96, img * 324 + dy * 18,
                                       [[18, 16], [1, 16]])
                            nc.tensor.matmul(ps2[64 * sp:64 * sp + 64, :],
                                             ct['W2'][:, dy * 64:(dy + 1) * 64], rhs,
                                             start=(dy == 0), stop=(dy == 2),
                                             tile_position=(0, 64 * sp))
                    t2 = t1p.tile([128, 64], f16, tag="t2")
                    nc.vector.tensor_reduce(
                        out=t2[:],
                        in_=view(ps2[:], 0, 128, 0, [[32, 8], [2, 8], [16, 2], [1, 2]]),
                        op=ALU.max, axis=AX.XY)
                    for sp in range(2):
                        img = 2 * p + sp
                        dstv = view(r3[:], 0, 64, img * 100 + 11, [[10, 8], [1, 8]])
                        nc.gpsimd.tensor_copy(dstv, view(t2[:], 64 * sp, 64, 0, [[8, 8], [1, 8]]))

                # bias+relu on conv2 pooled interior, then dx-shift copy
                r3int = view(r3[:], 0, 64, 11, [[100, 8], [10, 8], [1, 8]])
                nc.vector.tensor_scalar(r3int, r3int, ct['B2'][:], 0.0,
                                        op0=ALU.add, op1=ALU.max)
                c3src = bass.AP(tensor=r3[:].tensor, offset=r3[:].offset + 1,
                                ap=[[r3[:].ap[0][0], 64], [1, 800]])
                nc.gpsimd.dma_start(r3[64:128, 0:800], c3src)

                # ---- conv3 + relu + avgpool -> EF ----
                ps3 = ps3p.tile([128, 512], f32, tag="ps3")
                for dy in range(3):
                    rhsA = view(r3[:], 0, 128, dy * 10, [[100, 8], [10, 8], [1, 8]])
                    nc.tensor.matmul(ps3[:], ct['W3A'][:, dy * 128:(dy + 1) * 128],
                                     rhsA, start=(dy == 0), stop=False)
                    rhsB = view(r3[:], 0, 64, dy * 10 + 2, [[100, 8], [10, 8], [1, 8]])
                    nc.tensor.matmul(ps3[:], ct['W3B'][:, dy * 128:(dy + 1) * 128],
                                     rhsB, start=False, stop=(dy == 2))
                nc.scalar.activation(ps3[:], ps3[:], AF.Relu, bias=ct['B3'][:], scale=1.0)
                with nc.allow_low_precision("avgpool sums 4 values; fp16 out is fine"):
                    nc.vector.tensor_reduce(
                        out=view(ef[:], 0, 128, g * 128, [[16, 8], [4, 4], [1, 4]]),
                        in_=view(ps3[:], 0, 128, 0, [[64, 8], [16, 4], [2, 4], [8, 2], [1, 2]]),
                        op=ALU.add, axis=AX.XY)

        # ------------------- tail: fc1 / fc2 / quantum / head -------------------
        with tc.tile_pool(name="tsb", bufs=1) as tsb, \
             tc.tile_pool(name="psfp", bufs=1, space="PSUM") as psfp, \
             tc.tile_pool(name="pstp", bufs=2, space="PSUM") as pstp, \
             tc.tile_pool(name="tp1", bufs=2, space="PSUM") as tp1, \
             tc.tile_pool(name="tp2", bufs=2, space="PSUM") as tp2:

            psf = psfp.tile([nb, 512], f32, tag="psf")
            for s in range(16):
                lhsT = view(ef[:], 0, 128, s, [[16, nb]])
                nc.tensor.matmul(psf[:], lhsT,
                                 ct['W1FC'][:, s * 512:(s + 1) * 512],
                                 start=(s == 0), stop=False)
            nc.tensor.matmul(psf[:], ones_r[0:1, 0:nb], ct['B1FC'][:],
                             start=False, stop=True)
            h1t = tsb.tile([nb, 512], f32, tag="h1t")
            nc.scalar.activation(h1t[:], psf[:], AF.Relu)

            h1 = tsb.tile([128, 4 * nb], f32, tag="h1")
            for t in range(4):
                pst = pstp.tile([128, nb], f32, tag="pst")
                nc.tensor.transpose(pst[:], h1t[:, t * 128:(t + 1) * 128], ident[0:nb, 0:nb])
                nc.scalar.copy(h1[:, t * nb:(t + 1) * nb], pst[:])

            psz = tp1.tile([16, nb], f32, tag="tp1")
            for t in range(4):
                nc.tensor.matmul(psz[:], ct['W2FC'][:, t * 16:(t + 1) * 16],
                                 h1[:, t * nb:(t + 1) * nb],
                                 start=(t == 0), stop=(t == 3))
            e = tsb.tile([16, nb], f32, tag="e")
            nc.scalar.activation(e[:], psz[:], AF.Exp, bias=ct['B2FC'][:], scale=1.0)

            psr = tp2.tile([16, nb], f32, tag="tp2")
            nc.tensor.matmul(psr[:], ct['URT'][:], e[:], start=True, stop=True)
            psi2 = tp2.tile([16, nb], f32, tag="tp2")
            nc.tensor.matmul(psi2[:], ct['UIT'][:], e[:], start=True, stop=True)
            tr = tsb.tile([16, nb], f32, tag="tr")
            nc.scalar.square(tr[:], psr[:])
            ti = tsb.tile([16, nb], f32, tag="ti")
            nc.scalar.square(ti[:], psi2[:])
            pun = tsb.tile([16, nb], f32, tag="pun")
            nc.vector.tensor_add(pun[:], tr[:], ti[:])

            pss = tp1.tile([1, nb], f32, tag="tp1")
            nc.tensor.matmul(pss[:], ones_c[:], pun[:], start=True, stop=True)
            rec = tsb.tile([1, nb], f32, tag="rec")
            nc.vector.reciprocal(rec[:], pss[:])
            psb = tp1.tile([16, nb], f32, tag="tp1")
            nc.tensor.matmul(psb[:], ones_r[0:1, 0:16], rec[:], start=True, stop=True)
            bc = tsb.tile([16, nb], f32, tag="bc")
            nc.scalar.copy(bc[:], psb[:])
            pn = tsb.tile([16, nb], f32, tag="pn")
            nc.vector.tensor_mul(pn[:], pun[:], bc[:])

            psy = tp2.tile([128, nb], f32, tag="tp2")
            nc.tensor.matmul(psy[:], ct['WH'][:], pn[:], start=True, stop=True)
            h2 = tsb.tile([128, nb], f32, tag="h2")
            nc.scalar.activation(h2[:], psy[:], AF.Relu, bias=ct['CH'][:], scale=ct['AH'][:])

            pso = tp1.tile([nb, 100], f32, tag="tp1")
            nc.tensor.matmul(pso[:], h2[:], ct['H2WT'][:], start=True, stop=False)
            nc.tensor.matmul(pso[:], ones_r[0:1, 0:nb], ct['H2B'][:],
                             start=False, stop=True)
            outs = tsb.tile([nb, 100], f32, tag="outs")
            nc.scalar.copy(outs[:], pso[:])
            nc.sync.dma_start(OUT[:], outs[:])

    nc.finalize()
    return nc


def get_program(nb=NB):
    key = ("prog", nb)
    if key not in _cache:
        _cache[key] = _build_program(nb)
    return _cache[key]


# ---------------------------------------------------------------------------
# entry point
# ---------------------------------------------------------------------------

def kernel(**inputs):
    from concourse.bass_utils import run_bass_kernel_spmd

    x = np.asarray(inputs['x'], np.float32)
    B = x.shape[0]
    nb = B // NCORES
    hw = _host_weights({k: np.asarray(v) for k, v in inputs.items()})

    nc = get_program(nb)
    in_maps = []
    for c in range(NCORES):
        m = {'xr': _build_xr(x[c * nb:(c + 1) * nb])}
        m.update(hw)
        in_maps.append(m)
    res = run_bass_kernel_spmd(nc, in_maps, core_ids=list(range(NCORES)))
    return np.concatenate([res.results[c]['out'] for c in range(NCORES)], axis=0)

